# revision 1
# baseline (speedup 1.0000x reference)
"""Trainium2 Bass kernel for CompressionSDF (4,128,128,128) -> (4,128,128,128).

Structure of the computation:
  stage: 1x1-conv stack over (B,C=128,H,W): 128->64->32->16 (lrelu, lrelu, none)
  then per-voxel MLP over a z-broadcast 17-channel field: 17->32->32->16->1
  (lrelu x3, sigmoid), where channel 16 is a z linspace coordinate.

Sharding: H axis split across 8 cores (16 rows each). Per core 8192 pixels,
1,048,576 voxels.

Per-core kernel layout: voxels are packed 4-per-column: partition dim holds
4 z-groups x 32 channels; columns are (z_lo, pixel). Layer 1 is computed by a
"selector" matmul whose stationary operand holds the per-pixel conv features
(so the z-broadcast never materializes in HBM); layers 2/3/4 use block-diagonal
weight matrices. Leaky-relu evacuations ride the PSUM->SBUF copies
(DVE scalar_tensor_tensor for L1, ScalarE Prelu for L2/L3, Sigmoid for L4).
"""

import sys

sys.path.insert(0, "/opt/trn_rl_repo")

import numpy as np
from contextlib import ExitStack

import concourse.bass as bass
import concourse.tile as tile
from concourse import bacc, mybir
from concourse.bass_utils import run_bass_kernel_spmd

F32 = mybir.dt.float32
BF16 = mybir.dt.bfloat16
AF = mybir.ActivationFunctionType
ALU = mybir.AluOpType

N_CORES = 8
B, C, H, W, D = 4, 128, 128, 128, 128
HL = H // N_CORES            # h rows per core = 16
PIX = B * HL * W             # pixels per core = 8192
PB = 64                      # pixels per block
NBLK = PIX // PB             # 128 blocks
NCH = 4                      # chunks per block, 8 z_lo each -> 32 z_lo
ALPHA = 0.01                 # LeakyReLU slope

_CACHE = {}


def _build_program(trace=False):
    nc = bacc.Bacc(
        "TRN2",
        target_bir_lowering=False,
        debug=False,
        enable_asserts=False,
        num_devices=N_CORES,
    )

    def din(name, shape, dt=F32):
        return nc.dram_tensor(name, list(shape), dt, kind="ExternalInput")

    x_d = din("x_sb", (C, PIX), BF16)
    w1_d = din("w1T", (128, 64), BF16)
    b1_d = din("b1c", (64, 1))
    w2_d = din("w2T", (64, 32), BF16)
    b2_d = din("b2c", (32, 1))
    w3_d = din("w3T", (32, 16), BF16)
    b3_d = din("b3c", (16, 1))
    gw_d = din("gw", (17, 128), BF16)
    selrow_d = din("selrow", (2, 128), BF16)
    selrhs_d = din("selrhs", (66, NCH * 512), BF16)
    l2_d = din("l2T", (128, 128), BF16)
    b2t4_d = din("b2t4", (128, 1))
    l3_d = din("l3T", (128, 128), BF16)
    b3t8_d = din("b3t8", (128, 1))
    l4_d = din("l4T", (128, 8), BF16)
    mb4_d = din("mb4t8", (8, 1))
    out_d = nc.dram_tensor("out_sd", [D, PIX], F32, kind="ExternalOutput")

    with tile.TileContext(nc) as tc, ExitStack() as octx:
        cpool = octx.enter_context(tc.tile_pool(name="consts", bufs=1))

        def load(name, dram, shape, dt=F32):
            t = cpool.tile(list(shape), dt, name=name)
            nc.sync.dma_start(out=t[:], in_=dram[:])
            return t

        xt = load("xt", x_d, (C, PIX), BF16)
        w1s = load("w1s", w1_d, (128, 64), BF16)
        b1s = load("b1s", b1_d, (64, 1))
        w2s = load("w2s", w2_d, (64, 32), BF16)
        b2s = load("b2s", b2_d, (32, 1))
        w3s = load("w3s", w3_d, (32, 16), BF16)
        b3s = load("b3s", b3_d, (16, 1))
        gws = load("gws", gw_d, (17, 128), BF16)
        selrows = load("selrows", selrow_d, (2, 128), BF16)
        selrhss = load("selrhss", selrhs_d, (66, NCH * 512), BF16)
        l2s = load("l2s", l2_d, (128, 128), BF16)
        b2t4s = load("b2t4s", b2t4_d, (128, 1))
        l3s = load("l3s", l3_d, (128, 128), BF16)
        b3t8s = load("b3t8s", b3t8_d, (128, 1))
        l4s = load("l4s", l4_d, (128, 8), BF16)
        mb4s = load("mb4s", mb4_d, (8, 1))

        f1 = cpool.tile([64, PIX], BF16, name="f1")
        f2 = cpool.tile([32, PIX], BF16, name="f2")
        f3 = cpool.tile([17, PIX], BF16, name="f3")
        nc.vector.memset(f3[:], 1.0)  # row 16 stays 1.0; rows 0..15 overwritten

        # ---- stage: pointwise conv stack over pixels ----
        with tc.tile_pool(name="psA", bufs=2, space="PSUM") as psA, \
             tc.tile_pool(name="psB", bufs=2, space="PSUM") as psB, \
             tc.tile_pool(name="psC", bufs=2, space="PSUM") as psC:
            for t in range(PIX // 512):
                s = bass.ts(t, 512)
                pa = psA.tile([64, 512], F32, name="pa")
                nc.tensor.matmul(pa[:], w1s[:], xt[:, s], start=True, stop=True)
                nc.scalar.activation(f1[:, s], pa[:], AF.Prelu, bias=b1s[:], alpha=ALPHA)
                pb = psB.tile([32, 512], F32, name="pb")
                nc.tensor.matmul(pb[:], w2s[:], f1[:, s], start=True, stop=True)
                nc.scalar.activation(f2[:, s], pb[:], AF.Prelu, bias=b2s[:], alpha=ALPHA)
                pc = psC.tile([16, 512], F32, name="pc")
                nc.tensor.matmul(pc[:], w3s[:], f2[:, s], start=True, stop=True)
                nc.scalar.activation(f3[0:16, s], pc[:], AF.Identity, bias=b3s[:])

        # ---- per-voxel MLP ----
        # z row index = 32*zg + t, t = 8*chunk + j
        osd = out_d[:].rearrange("(zg t) n -> zg t n", zg=4)

        with tc.tile_pool(name="ps1", bufs=2, space="PSUM") as ps1, \
             tc.tile_pool(name="ps2", bufs=2, space="PSUM") as ps2, \
             tc.tile_pool(name="ps3", bufs=2, space="PSUM") as ps3, \
             tc.tile_pool(name="ps4", bufs=2, space="PSUM") as ps4, \
             tc.tile_pool(name="hpool", bufs=5) as hpool, \
             tc.tile_pool(name="lhsp", bufs=3) as lhsp, \
             tc.tile_pool(name="sigp", bufs=3) as sigp:
            for blk in range(NBLK):
                bs = bass.ts(blk, PB)
                # build the selector stationary operand for this pixel block:
                # rows 0..63 = per-pixel conv features g (replicated x4 over
                # z-groups, via a small matmul), rows 64..65 = z-coordinate rows
                pg = ps1.tile([PB, 128], F32, name="pg", tag="pre1")
                nc.tensor.matmul(pg[:], f3[:, bs], gws[:], start=True, stop=True)
                lhsTb = lhsp.tile([66, 128], BF16, name="lhsTb")
                nc.vector.tensor_copy(lhsTb[0:PB, :], pg[:])
                nc.sync.dma_start(out=lhsTb[PB:PB + 2, :], in_=selrows[:])

                p3 = None
                psig = None
                for c in range(NCH):
                    u, q = c // 2, c % 2
                    # L1: selector matmul -> pre1; DVE lrelu evac
                    p1 = ps1.tile([128, 512], F32, name="p1", tag="pre1")
                    nc.tensor.matmul(p1[:], lhsTb[:], selrhss[:, bass.ts(c, 512)],
                                     start=True, stop=True)
                    h1 = hpool.tile([128, 512], BF16, name="h1")
                    nc.scalar.activation(h1[:], p1[:], AF.Prelu, alpha=ALPHA)
                    # L2: block-diag matmul; DVE 2-pass lrelu evac:
                    # h2 = p2 - 0.99*min(p2+b2, 0) = lrelu(p2+b2) - b2,
                    # with the missing b2 folded into L3's bias host-side.
                    p2 = ps2.tile([128, 512], F32, name="p2")
                    nc.tensor.matmul(p2[:], l2s[:], h1[:], start=True, stop=True)
                    m2 = hpool.tile([128, 512], F32, name="m2")
                    nc.vector.tensor_scalar(m2[:], p2[:], b2t4s[:], 0.0,
                                            op0=ALU.add, op1=ALU.min)
                    h2 = hpool.tile([128, 512], BF16, name="h2")
                    nc.vector.scalar_tensor_tensor(h2[:], m2[:], -0.99, p2[:],
                                                   op0=ALU.mult, op1=ALU.add)
                    # L3: two chunks share one PSUM tile via column groups
                    if q == 0:
                        p3 = ps3.tile([128, 512], F32, name="p3")
                    nc.tensor.matmul(p3[q * 64:(q + 1) * 64, :],
                                     l3s[:, q * 64:(q + 1) * 64], h2[:],
                                     start=True, stop=True,
                                     tile_position=(0, q * 64))
                    if q == 1:
                        h3 = hpool.tile([128, 512], BF16, name="h3")
                        nc.scalar.activation(h3[:], p3[:], AF.Prelu,
                                             bias=b3t8s[:], alpha=ALPHA)
                        # L4: 8 output rows = (chunk-in-pair, z-group)
                        psig = ps4.tile([8, 512], F32, name="psig")
                        nc.tensor.matmul(psig[:], l4s[:], h3[:],
                                         start=True, stop=True)
                        sig = sigp.tile([8, 512], F32, name="sig")
                        nc.scalar.activation(sig[:], psig[:], AF.Sigmoid,
                                             bias=mb4s[:])
                        for qq in range(2):
                            cc = 2 * u + qq
                            # z = 32*zg + 8*cc + j ; sig row 4*qq+zg
                            src = sig[4 * qq:4 * qq + 4, :]
                            src = src.rearrange("p (j w) -> p j w", j=8)
                            dst = osd[:, 8 * cc:8 * (cc + 1), bs]
                            nc.sync.dma_start(out=dst, in_=src)

    nc.compile()
    return nc


def _host_inputs(x, sw1, sb1, sw2, sb2, sw3, sb3,
                 mw1, mb1, mw2, mb2, mw3, mb3, mw4, mb4):
    import ml_dtypes
    f = np.float32
    bf = ml_dtypes.bfloat16
    zt = np.linspace(-1.0, 1.0, D, dtype=np.float64)
    c1 = mw1[:, 16].astype(np.float64)
    W1f = mw1[:, :16]

    gw = np.zeros((17, 128), f)
    gw[:16, :] = np.tile(W1f.T, (1, 4))
    gw[16, :] = np.tile(mb1, 4)

    A = zt[::32]                      # z-group base coordinate, shape (4,)
    Bv = zt[:32] - zt[0]              # z_lo offset, shape (32,)
    selrow = np.zeros((2, 128), f)
    selrow[0] = np.repeat(A, 32) * np.tile(c1, 4)
    selrow[1] = np.tile(c1, 4)

    selrhs = np.zeros((66, NCH * 512), f)
    eye_tiled = np.tile(np.eye(PB, dtype=f), (1, 8))   # [64, 512], col = j*64+p
    for c in range(NCH):
        s = slice(c * 512, (c + 1) * 512)
        selrhs[:PB, s] = eye_tiled
        selrhs[PB, s] = 1.0
        selrhs[PB + 1, s] = np.repeat(Bv[8 * c:8 * c + 8], PB)

    ins = {
        "w1T": np.ascontiguousarray(sw1.T).astype(bf),
        "b1c": sb1[:, None].astype(f),
        "w2T": np.ascontiguousarray(sw2.T).astype(bf),
        "b2c": sb2[:, None].astype(f),
        "w3T": np.ascontiguousarray(sw3.T).astype(bf),
        "b3c": sb3[:, None].astype(f),
        "gw": gw.astype(bf),
        "selrow": selrow.astype(bf),
        "selrhs": selrhs.astype(bf),
        "l2T": np.kron(np.eye(4, dtype=f), mw2.T).astype(bf),
        "b2t4": np.tile(mb2, 4)[:, None].astype(f),
        "l3T": np.concatenate([np.kron(np.eye(4, dtype=f), mw3.T)] * 2,
                              axis=1).astype(bf),
        "b3t8": np.tile(mb3 + mw3 @ mb2, 8)[:, None].astype(f),
        "l4T": np.kron(np.eye(8, dtype=f), mw4.T).astype(bf),
        "mb4t8": np.full((8, 1), mb4[0], f),
    }
    in_maps = []
    for k in range(N_CORES):
        xs = x[:, :, k * HL:(k + 1) * HL, :]
        xcore = np.ascontiguousarray(
            xs.transpose(1, 0, 2, 3).reshape(C, PIX)).astype(bf)
        in_maps.append({**ins, "x_sb": xcore})
    return in_maps


def run(trace=False, **inputs):
    if "nc" not in _CACHE:
        _CACHE["nc"] = _build_program()
    nc = _CACHE["nc"]
    in_maps = _host_inputs(**inputs)
    res = run_bass_kernel_spmd(nc, in_maps, list(range(N_CORES)), trace=trace)
    out = np.empty((B, D, H, W), np.float32)
    for k in range(N_CORES):
        o = res.results[k]["out_sd"].reshape(D, B, HL, W).transpose(1, 0, 2, 3)
        out[:, :, k * HL:(k + 1) * HL, :] = o
    return out, res


def kernel(**inputs):
    out, _ = run(trace=False, **inputs)
    return out

